# revision 3
# baseline (speedup 1.0000x reference)
"""Trainium2 Bass kernel for nn_CMAttention (Infini-attention with compressive memory).

Sharding: 8 cores = 2 (batch) x 4 (head-groups of 4 heads). Each core computes the
full packed sequence [audio(2048); x(2048)] for its batch and its 4 heads.

v3: fp16 datapath (2x DVE, halved DMA, fp16 output with host upcast),
software-pipelined segments: segment s+1's projection (PE-bound, ACT-idle)
is interleaved into segment s's attention phase (ACT-bound exp, PE ~60%),
with separate PSUM tags so the projection isn't gated on the exp drain.
Engine placement: Pool takes PSUM evacuations + norm-scale mult, ACT takes
the qkT copies (its phase-1 window is idle) plus all exps, DVE keeps the
2-byte 2x/4x ops (rope, elu, masks, combines).
"""
from contextlib import ExitStack

import numpy as np

# problem dims (hardcoded per contract)
DIM = 1024
HEADS = 16
DH = 64
SEG = 1024
B = 2
NA = 2048
NX = 2048
NTOT = NA + NX
N_CORES = 8
HL = 4              # heads per core
NSEG = NTOT // SEG  # 4
ST = SEG // 128     # 8 seq tiles per segment
KC = DIM // 128     # 8 contraction chunks

_PERM = np.concatenate([np.arange(0, DH, 2), np.arange(1, DH, 2)])  # evens, odds
_LN8 = float(np.log(8.0))


def build_program():
    import concourse.tile as tile
    import concourse.mybir as mybir
    from concourse import bacc
    from concourse.masks import make_identity, make_upper_triangular

    dt = mybir.dt
    AF = mybir.ActivationFunctionType
    ALU = mybir.AluOpType
    AX = mybir.AxisListType

    import concourse.mybir as _mb
    import bass_rust as _br
    from concourse.hw_specs import get_activation_tables as _gat

    class _Bacc(bacc.Bacc):
        def insert_act_table_loads(self):
            has_act = any(isinstance(i, _mb.InstActivation)
                          for b_ in self.main_func.blocks
                          for i in b_.instructions)
            if not has_act:
                return
            keep = "natural_log_exp_and_others"
            tables = [(nm, fns if nm == keep else set())
                      for nm, fns in _gat(self.m.arch).items()]
            _br.insert_act_table_loads(self, tables)

    nc = _Bacc("TRN2", target_bir_lowering=False, debug=False,
               num_devices=N_CORES, name="cmattn")

    # ---- DRAM I/O (per core) ----
    at_d = nc.dram_tensor("at16", (DIM, NA), dt.float16, kind="ExternalInput")
    xt_d = nc.dram_tensor("xt16", (DIM, NX), dt.float16, kind="ExternalInput")
    wa_d = nc.dram_tensor("wa", (DIM, 3 * HL * DH), dt.float16, kind="ExternalInput")
    wx_d = nc.dram_tensor("wx", (DIM, 3 * HL * DH), dt.float16, kind="ExternalInput")
    cos_d = nc.dram_tensor("cos2", (128, NSEG * ST, DH), dt.float16, kind="ExternalInput")
    sin_d = nc.dram_tensor("sin2", (128, NSEG * ST, DH), dt.float16, kind="ExternalInput")
    g_d = nc.dram_tensor("gvec", (1, HL), dt.float32, kind="ExternalInput")
    oa_d = nc.dram_tensor("out_a", (NA, HL * DH), dt.float16, kind="ExternalOutput")
    ox_d = nc.dram_tensor("out_x", (NX, HL * DH), dt.float16, kind="ExternalOutput")

    oa_r = oa_d.ap().rearrange("(t p) c -> p t c", p=128)   # [128, 16, 256]
    ox_r = ox_d.ap().rearrange("(t p) c -> p t c", p=128)

    with tile.TileContext(nc) as tc, ExitStack() as ctx:
        cpool = ctx.enter_context(tc.tile_pool(name="const", bufs=1))
        wpool = ctx.enter_context(tc.tile_pool(name="wpool", bufs=2))
        spool = ctx.enter_context(tc.tile_pool(name="spool", bufs=2))
        spool2 = ctx.enter_context(tc.tile_pool(name="spool2", bufs=2))
        ppool = ctx.enter_context(tc.tile_pool(name="ppool", bufs=2))
        tpool = ctx.enter_context(tc.tile_pool(name="tpool", bufs=2))
        bpool = ctx.enter_context(tc.tile_pool(name="bpool", bufs=2))
        psQ = ctx.enter_context(tc.tile_pool(name="psQ", bufs=2, space="PSUM"))
        psW = ctx.enter_context(tc.tile_pool(name="psW", bufs=2, space="PSUM"))
        psB = ctx.enter_context(tc.tile_pool(name="psB", bufs=2, space="PSUM"))

        # ---- constants ----
        ident_f = cpool.tile([128, 128], dt.float32, tag="identf")
        make_identity(nc, ident_f[:])
        ident = cpool.tile([128, 128], dt.float16, tag="ident")
        nc.gpsimd.tensor_copy(ident[:], ident_f[:])
        tri01 = cpool.tile([128, 128], dt.float16, tag="tri")
        make_upper_triangular(nc, tri01[:], val=1.0, diag=True)  # 1 where k<=q
        g_sb = cpool.tile([128, HL], dt.float32, tag="g")
        nc.sync.dma_start(g_sb[:], g_d.ap().to_broadcast((128, HL)))
        omg_sb = cpool.tile([128, HL], dt.float32, tag="omg")
        nc.vector.tensor_scalar(omg_sb[:], g_sb[:], -1.0, 1.0, ALU.mult, ALU.add)
        M16 = cpool.tile([128, HL, DH + 1], dt.float16, tag="M16")
        nc.vector.memset(M16[:], 0.0)
        ln8_sb = cpool.tile([128, 1], dt.float32, tag="ln8")
        nc.vector.memset(ln8_sb[:], _LN8)

        S = [None] * NSEG   # per-segment tile dicts
        W = [None, None]    # per-source weights (0: audio, 1: x)

        def alloc_seg(seg):
            d = {}
            col0 = (seg % 2) * SEG
            if seg in (0, 2):
                w_sb = wpool.tile([128, KC, 3 * HL * DH], dt.float16, tag="wsb")
                w_src = wa_d if seg == 0 else wx_d
                for kc in range(KC):
                    nc.sync.dma_start(w_sb[:, kc, :],
                                      w_src.ap()[kc * 128:(kc + 1) * 128, :])
                W[seg // 2] = w_sb
            d["w"] = W[seg // 2]

            src16 = at_d if seg < 2 else xt_d
            xt_sb = spool.tile([128, KC, SEG], dt.float16, tag="xt")
            for kc in range(KC):
                nc.sync.dma_start(
                    xt_sb[:, kc, :],
                    src16.ap()[kc * 128:(kc + 1) * 128, col0:col0 + SEG])
            d["xt"] = xt_sb

            cos_sb = spool.tile([128, ST, DH], dt.float16, tag="cos")
            sin_sb = spool.tile([128, ST, DH], dt.float16, tag="sin")
            nc.sync.dma_start(cos_sb[:], cos_d.ap()[:, seg * ST:(seg + 1) * ST, :])
            nc.sync.dma_start(sin_sb[:], sin_d.ap()[:, seg * ST:(seg + 1) * ST, :])
            d["cos"], d["sin"] = cos_sb, sin_sb

            d["qkT"] = spool2.tile([128, 2, 2, SEG], dt.float16, tag="qkT", name="qkT")
            d["sqT"] = spool2.tile([128, 2, SEG], dt.float16, tag="sqT", name="sqT")
            d["sk"] = spool2.tile([128, ST, HL, 2, DH], dt.float16, tag="skal", name="sk")
            d["v16"] = spool2.tile([128, ST, HL, DH + 1], dt.float16, tag="v16", name="v16")
            nc.gpsimd.memset(d["v16"][:, :, :, DH], 1.0)  # ones column
            d["outst"] = spool2.tile([128, ST, HL, DH], dt.float16, tag="outst", name="outst")
            d["sro"] = spool2.tile([128, ST, 2 * HL, DH], dt.float16, tag="sro", name="sro")
            d["ss"] = spool2.tile([128, ST, 2 * HL], dt.float32, tag="ss", name="ss")
            d["rsc"] = spool2.tile([128, ST, 2 * HL], dt.float32, tag="rsc", name="rsc")
            S[seg] = d

        def phase1_tile(seg, st):
            d = S[seg]
            qk_ps = psQ.tile([128, 512], dt.float32, tag="qk")
            v_ps = psB.tile([128, 256], dt.float32, tag="sm")
            for kc in range(KC):
                lhsT = d["xt"][:, kc, st * 128:(st + 1) * 128]
                nc.tensor.matmul(qk_ps[:], lhsT=lhsT,
                                 rhs=d["w"][:, kc, 0:512],
                                 start=(kc == 0), stop=(kc == KC - 1))
                nc.tensor.matmul(v_ps[:], lhsT=lhsT,
                                 rhs=d["w"][:, kc, 512:768],
                                 start=(kc == 0), stop=(kc == KC - 1))

            # PSUM evacuation on Pool
            qk_sb = tpool.tile([128, 2 * HL, DH], dt.float16, tag="qksb")
            nc.scalar.copy(
                qk_sb[:], qk_ps[:].rearrange("p (h d) -> p h d", h=2 * HL))
            nc.scalar.copy(
                d["v16"][:, st, :, 0:DH],
                v_ps[:].rearrange("p (h d) -> p h d", h=HL))

            # rope (fp16, 2x DVE): sro = qk*cos + rot(qk)*sin
            sro = d["sro"]
            cosb = d["cos"][:, st:st + 1, :].to_broadcast((128, 2 * HL, DH))
            sinb = d["sin"][:, st:st + 1, :].to_broadcast((128, 2 * HL, DH))
            nc.vector.tensor_tensor(sro[:, st], qk_sb[:], cosb, ALU.mult)
            rt2 = bpool.tile([128, 2 * HL, DH], dt.float16, tag="rt2")
            rot = qk_sb[:].rearrange("p h (u d) -> p h u d", u=2)[:, :, ::-1, :]
            nc.vector.tensor_tensor(
                rt2[:].rearrange("p h (u d) -> p h u d", u=2), rot,
                sinb.rearrange("p h (u d) -> p h u d", u=2), ALU.mult)
            nc.vector.tensor_add(sro[:, st], sro[:, st], rt2[:])
            # sum of squares per head (rope preserves norms -> use raw qk)
            sq2 = bpool.tile([128, 2 * HL, DH], dt.float16, tag="sq2")
            nc.vector.tensor_tensor(sq2[:], qk_sb[:], qk_sb[:], ALU.mult)
            nc.vector.tensor_reduce(d["ss"][:, st], sq2[:], axis=AX.X, op=ALU.add)

        def phase1_tail(seg):
            d = S[seg]
            # batched rmsnorm scale: rsc = 8/sqrt(ss) = exp(-0.5*ln(ss)+ln8)
            lnv = tpool.tile([128, ST, 2 * HL], dt.float32, tag="lnv")
            nc.scalar.activation(lnv[:], d["ss"][:], AF.Ln)
            nc.scalar.activation(d["rsc"][:], lnv[:], AF.Exp, scale=-0.5,
                                 bias=ln8_sb[:, 0:1])
            sro, qkT = d["sro"], d["qkT"]
            for st in range(ST):
                nc.gpsimd.tensor_tensor(
                    sro[:, st], sro[:, st],
                    d["rsc"][:, st, :, None].to_broadcast((128, 2 * HL, DH)),
                    ALU.mult)
                tp_ps = psQ.tile([128, 512], dt.float16, tag="qk")
                flat = sro[:, st].rearrange("p h d -> p (h d)")
                for i in range(4):
                    nc.tensor.transpose(tp_ps[:, i * 128:(i + 1) * 128],
                                        flat[:, i * 128:(i + 1) * 128], ident[:])
                nc.vector.tensor_copy(
                    qkT[:, :, :, st * 128:(st + 1) * 128],
                    tp_ps[:].rearrange("p (a b s) -> p a b s", a=2, b=2))

            # sq = elu(qT)+1 (transposed layout)
            el1 = bpool.tile([128, 2, SEG], dt.float16, tag="el1")
            nc.vector.tensor_scalar_min(el1[:], qkT[:, 0], 0.0)
            nc.scalar.activation(d["sqT"][:], el1[:], AF.Exp)
            nc.vector.tensor_scalar_max(el1[:], qkT[:, 0], 0.0)
            nc.vector.tensor_add(d["sqT"][:], d["sqT"][:], el1[:])

            # sk = elu(kn)+1 (token layout, duplicated for the M update)
            kn_v = sro[:, :, HL:2 * HL, :]
            ska = bpool.tile([128, ST, HL, DH], dt.float16, tag="el2")
            sk0 = d["sk"][:, :, :, 0, :]
            nc.vector.tensor_scalar_min(ska[:], kn_v, 0.0)
            nc.scalar.activation(sk0, ska[:], AF.Exp)
            nc.vector.tensor_scalar_max(ska[:], kn_v, 0.0)
            nc.vector.tensor_add(sk0, sk0, ska[:])
            nc.scalar.copy(d["sk"][:, :, :, 1, :], sk0)

        def phase2_head(seg, h):
            d = S[seg]
            pi, po = h >> 1, (h & 1) * 64
            kTh = d["qkT"][po:po + 64, 1, pi, :]
            qTh = d["qkT"][po:po + 64, 0, pi, :]

            # scores -> P = exp(S/8) in fp16, [k, q] layout
            P16 = ppool.tile([128, KC, SEG], dt.float16, tag="P16")
            for qc in range(4):
                ktn = 2 * qc + 2
                for kt0 in range(0, ktn, 4):
                    nkt = min(4, ktn - kt0)
                    wv = psW.tile([128, 4, 256], dt.float32, tag="wv")
                    for i in range(nkt):
                        kt = kt0 + i
                        nc.tensor.matmul(
                            wv[:, i, :],
                            lhsT=kTh[:, kt * 128:(kt + 1) * 128],
                            rhs=qTh[:, qc * 256:(qc + 1) * 256],
                            start=True, stop=True)
                    nc.scalar.activation(
                        P16[:, kt0:kt0 + nkt, qc * 256:(qc + 1) * 256],
                        wv[:, 0:nkt, :], AF.Exp, scale=0.125)
                # both diagonal blocks in one op via a hand-built
                # [[1152, 2], [1, 128]] strided view of P16
                b1 = (2 * qc) * SEG + qc * 256
                dgv = P16[:].rearrange("p a b -> p (a b)")[
                    :, b1:b1 + 1153:1152][:, :, None].copy()
                dgv.ap[2] = [1, 128]
                nc.vector.tensor_tensor(
                    dgv, dgv,
                    tri01[:, None, :].to_broadcast((128, 2, 128)),
                    ALU.mult)

            # AV + memory retrieval + combine, in groups of 4 q-tiles
            for g2 in range(2):
                av_ps = psB.tile([128, 4, DH + 1], dt.float32, tag="sm",
                                 name="av_ps")
                if seg > 0:
                    mem_ps = psB.tile([128, 4, DH + 1], dt.float32, tag="sm",
                                      name="mem_ps")
                for qi in range(4):
                    qt = 4 * g2 + qi
                    for kt in range(qt + 1):
                        nc.tensor.matmul(
                            av_ps[:, qi, :],
                            lhsT=P16[:, kt, qt * 128:(qt + 1) * 128],
                            rhs=d["v16"][:, kt, h, :],
                            start=(kt == 0), stop=(kt == qt))
                    if seg > 0:
                        nc.tensor.matmul(
                            mem_ps[:, qi, :],
                            lhsT=d["sqT"][po:po + 64, pi, qt * 128:(qt + 1) * 128],
                            rhs=M16[po:po + 64, h, :],
                            start=True, stop=True)
                # combine into staging
                rl = tpool.tile([128, 4], dt.float32, tag="rl")
                nc.vector.reciprocal(rl[:], av_ps[:, :, DH])
                nc.vector.tensor_scalar_mul(rl[:], rl[:], omg_sb[:, h:h + 1])
                loc = d["outst"][:, 4 * g2:4 * g2 + 4, h, :]
                nc.vector.tensor_tensor(
                    loc, av_ps[:, :, 0:DH],
                    rl[:, :, None].to_broadcast((128, 4, DH)), ALU.mult)
                if seg > 0:
                    rm = tpool.tile([128, 4], dt.float32, tag="rm")
                    nc.vector.tensor_scalar_add(rm[:], mem_ps[:, :, DH], 1e-6)
                    nc.vector.reciprocal(rm[:], rm[:])
                    nc.vector.tensor_scalar_mul(rm[:], rm[:], g_sb[:, h:h + 1])
                    cmb = tpool.tile([128, 4, DH], dt.float16, tag="cmb")
                    nc.vector.tensor_tensor(
                        cmb[:], mem_ps[:, :, 0:DH],
                        rm[:, :, None].to_broadcast((128, 4, DH)), ALU.mult)
                    nc.gpsimd.tensor_add(loc, loc, cmb[:])

            # memory update (after retrieval reads of this segment)
            mu_ps = psB.tile([128, DH + 1], dt.float32, tag="sm")
            for st2 in range(ST):
                nc.tensor.matmul(
                    mu_ps[:],
                    lhsT=d["sk"][:, st2, h, :, :].rearrange("p u d -> p (u d)"),
                    rhs=d["v16"][:, st2, h, :],
                    start=(st2 == 0), stop=(st2 == ST - 1))
            nc.vector.tensor_add(M16[:, h, :], M16[:, h, :], mu_ps[:])

        def out_dma(seg):
            out_r = oa_r if seg < 2 else ox_r
            t0 = (seg % 2) * ST
            nc.sync.dma_start(
                out_r[:, t0:t0 + ST, :],
                S[seg]["outst"][:].rearrange("p t h d -> p t (h d)"))

        # ---- software-pipelined emission ----
        alloc_seg(0)
        for seg in range(NSEG):
            if seg + 1 < NSEG:
                alloc_seg(seg + 1)          # prefetch DMAs early
            for h in range(HL):
                if seg > 0:
                    phase2_head(seg - 1, h)
                phase1_tile(seg, 2 * h)
                phase1_tile(seg, 2 * h + 1)
            if seg > 0:
                out_dma(seg - 1)
            phase1_tail(seg)
        for h in range(HL):
            phase2_head(NSEG - 1, h)
        out_dma(NSEG - 1)

    nc.compile()
    return nc


def prep_core_inputs(x, a, W_qkv_x, W_qkv_a, g_qx, g_kx, g_qa, g_ka, gate):
    """Host-side sharding: returns list of per-core input dicts."""
    x = np.asarray(x, np.float32)
    a = np.asarray(a, np.float32)
    W_qkv_x = np.asarray(W_qkv_x, np.float32)
    W_qkv_a = np.asarray(W_qkv_a, np.float32)
    gate = np.asarray(gate, np.float32)
    for gm in (g_qx, g_kx, g_qa, g_ka):
        assert np.allclose(np.asarray(gm), 1.0), "non-unit gamma not supported"

    # rope tables (global positions over packed [a; x])
    pos = np.arange(NTOT, dtype=np.float64)
    inv_freq = 1.0 / (10000.0 ** (np.arange(0, DH, 2, dtype=np.float64) / DH))
    ang = pos[:, None] * inv_freq[None, :]
    c, s = np.cos(ang), np.sin(ang)
    cos2 = np.concatenate([c, c], axis=1).astype(np.float16)      # [NTOT, 64]
    sin2 = np.concatenate([-s, s], axis=1).astype(np.float16)
    cos_t = np.ascontiguousarray(cos2.reshape(NSEG * ST, 128, DH).transpose(1, 0, 2))
    sin_t = np.ascontiguousarray(sin2.reshape(NSEG * ST, 128, DH).transpose(1, 0, 2))

    def wslice(W, heads):
        qs = [W[:, 64 * h + _PERM] for h in heads]
        ks = [W[:, DIM + 64 * h + _PERM] for h in heads]
        vs = [W[:, 2 * DIM + 64 * h:2 * DIM + 64 * h + 64] for h in heads]
        return np.ascontiguousarray(
            np.concatenate(qs + ks + vs, axis=1)).astype(np.float16)

    gsig = (1.0 / (1.0 + np.exp(-gate.astype(np.float64)))).astype(np.float32)

    at16 = [np.ascontiguousarray(a[b].T).astype(np.float16) for b in range(B)]
    xt16 = [np.ascontiguousarray(x[b].T).astype(np.float16) for b in range(B)]

    ins = []
    for c_ in range(N_CORES):
        b, hg = divmod(c_, 4)
        heads = [4 * hg + i for i in range(HL)]
        ins.append({
            "at16": at16[b],
            "xt16": xt16[b],
            "wa": wslice(W_qkv_a, heads),
            "wx": wslice(W_qkv_x, heads),
            "cos2": cos_t,
            "sin2": sin_t,
            "gvec": np.ascontiguousarray(gsig[heads])[None, :],
        })
    return ins


def assemble_outputs(results):
    out_x = np.empty((B, NX, DIM), np.float32)
    out_a = np.empty((B, NA, DIM), np.float32)
    for c_ in range(N_CORES):
        b, hg = divmod(c_, 4)
        out_x[b, :, 256 * hg:256 * (hg + 1)] = results[c_]["out_x"]
        out_a[b, :, 256 * hg:256 * (hg + 1)] = results[c_]["out_a"]
    return out_x, out_a


_PROGRAM_CACHE = {}


def get_program():
    if "nc" not in _PROGRAM_CACHE:
        _PROGRAM_CACHE["nc"] = build_program()
    return _PROGRAM_CACHE["nc"]


def kernel(**inputs):
    from concourse import bass_utils
    nc = get_program()
    ins = prep_core_inputs(**inputs)
    res = bass_utils.run_bass_kernel_spmd(nc, ins, core_ids=list(range(N_CORES)))
    return assemble_outputs(res.results)
